# revision 27
# baseline (speedup 1.0000x reference)
"""Trainium2 Bass kernel for nn_Attention_87625922773171.

Spatial-reduction attention (PVT-style) over B=4, N=5120 (1024 template +
4096 search tokens), C=256, 8 heads, sr_ratio=2.

Sharding: 8 cores = 4 batches x 2 head-groups (4 heads each).  Each core
computes its (b, hg) slice end-to-end in a transposed ("channels on
partitions") layout; the host sums the two head-group partial projections,
adds bproj, and transposes back.

Attention loop: software-pipelined.  The S PSUM tiles rotate through a
single 3-slot pool shared by the ACT and DVE exp consumers, so each slot's
reuse lands ~1.5 iterations after its exp started and neither exp engine
ever blocks an S matmul.  O/den matmuls trail S/exp by one j-iteration and
are grouped by stationary operand so the 4 col-banded tiles run
concurrently on the PE sub-arrays.

Device dataflow per core (all layouts transposed: [channels, tokens]):
  xT [256,5120] --f32r matmul--> Q^T (fp16)
  xT --strided-AP conv matmul--> y^T --LN (stats via ones-matmul)--> cat^T (fp16)
  cat^T --fp16 matmuls--> K^T [128,1280], V [1280,128]
  per 512-query tile: S^T = K^T_h.T @ Q^T_h (row-tiled 4 heads, fp16)
      P = exp(scale*S^T)  (ACT heads 0-1, DVE poly heads 2-3, fp16)
      O += V_h.T @ P_h; den += ones.T @ P_h (col-tiled, PSUM accumulate)
      O_norm = O * recip(den);  y^T = Wproj_part.T @ O_norm (fp32)
"""
import os
import contextlib
import numpy as np

import concourse.bacc as bacc
import concourse.mybir as mybir
import concourse.tile as tile
from concourse.bass_utils import run_bass_kernel_spmd

F32 = mybir.dt.float32
F32R = mybir.dt.float32r
F16 = mybir.dt.float16
AF = mybir.ActivationFunctionType
OP = mybir.AluOpType

B, N, C = 4, 5120, 256
NHEADS, D, SR = 8, 32, 2
HZ = WZ = 32
HX = WX = 64
NZ, NX = HZ * WZ, HX * WX  # 1024, 4096
LZ, LX = (HZ // SR) * (WZ // SR), (HX // SR) * (WX // SR)  # 256, 1024
L = LZ + LX  # 1280
SCALE = float(D) ** -0.5
EPS = 1e-5
NCORES = 8
QTILE = 512
NQT = N // QTILE            # 10 query tiles (0,1 are template queries)
NJT = L // 128              # 10 key tiles (0,1 are template keys)
ZQT = NZ // QTILE           # 2
ZJT = LZ // 128             # 2

_CACHED = {}

# degree-4 polynomial exp(SCALE*s) on s in [-4.59, 4.59] (scaled logits in
# [-0.81, 0.81], measured range +2%); p(0)=1 constrained LSQ on relative
# error, max rel err 4.3e-4 at range edge.  Coefficients folded with SCALE.
_EC = (0.99932575, 0.50072616, 0.17232145, 0.04077664)
EXP_C1 = _EC[0] * SCALE
EXP_C2 = _EC[1] * SCALE ** 2
EXP_C3 = _EC[2] * SCALE ** 3
EXP_C4 = _EC[3] * SCALE ** 4


def _register_exp_op():
    import concourse.dve_ops as dvo
    from concourse.dve_spec import (
        Spec, Src0, One, C0, C1, C2, C3, _spill_c3_to_src1, _has_src1, lower)
    from concourse.dve_uop import DveOpSpec
    name = "ANT_EXP_POLY4"
    for op in dvo.OPS:
        if op.name == name:
            return op
    body = _spill_c3_to_src1(
        One + Src0 * (C0 + Src0 * (C1 + Src0 * (C2 + Src0 * C3))))

    def _ref(in0, in1, s0, s1, imm2):
        c3 = np.asarray(in1).reshape(in1.shape[0], -1)[:, :1]
        return 1.0 + in0 * (s0 + in0 * (s1 + in0 * (imm2 + in0 * c3)))

    spec = Spec(body=body, reference=_ref)
    dvo._SUB_OPCODE_FOR_NAME[name] = dvo._CUSTOM_DVE_ROW_BASE + len(dvo.OPS)
    shas = {}
    for ver in ("v3", "v4"):
        s = DveOpSpec(name=name, opcode=dvo.get_dve_sub_opcode(name),
                      uops=lower(spec, ver=ver), rd1_en=_has_src1(spec))
        shas[ver] = s.sha(ver)
    op = dvo.DveOp(name, spec, subdim=False, uops_sha=shas)
    dvo.OPS.append(op)
    dvo.CUSTOM_DVE_SPECS[name] = spec
    return op


EXP_OP = _register_exp_op()

PHASES = os.environ.get("PHASES", "all")  # all | pre | attn
MS_ZERO = os.environ.get("MS_ZERO", "act")  # dve | act | pe (acc zeroing engine)
S_BUFS = int(os.environ.get("S_BUFS", "3"))  # shared S-psum rotation slots
EXPF = os.environ.get("EXPF", "full")  # full | mini (probe: tiny exp slices)
NODEN = os.environ.get("NODEN", "0") == "1"  # probe: skip den matmuls
BIG2 = int(os.environ.get("BIG2", "2"))  # big-tensor bufs (2 = cross-body dbuf)
PSPLIT = os.environ.get("PSPLIT", "1") == "1"  # proj evac on ACT+DVE
DMA_CHUNK = int(os.environ.get("DMA_CHUNK", "2560"))  # xT DMA chunk (0 = off)
ODEN_LAG = int(os.environ.get("ODEN_LAG", "2"))  # O/den trail distance (jts)
DEFER_TAIL = os.environ.get("DEFER_TAIL", "1") == "1"  # qt tail into next qt
TAIL_DUMMY = os.environ.get("TAIL_DUMMY", "0") == "1"  # keep s-slot parity
EMIT_MID = os.environ.get("EMIT_MID", "0") == "1"  # O/den between S_a and S_b
RECIP_C = 1.0 / C


def _build_nc(repeat=1):
    nc = bacc.Bacc("TRN2", target_bir_lowering=False)

    xT_d = nc.declare_dram_parameter("xT", [C, N], F16, isOutput=False)
    wq_d = nc.declare_dram_parameter("wq", [C, 128], F16, isOutput=False)
    wsr_d = nc.declare_dram_parameter("wsr", [8, 128, C], F16, isOutput=False)
    wk_d = nc.declare_dram_parameter("wk", [C, 128], F16, isOutput=False)
    wv_d = nc.declare_dram_parameter("wv", [C, 128], F16, isOutput=False)
    wp_d = nc.declare_dram_parameter("wp", [128, C], F32, isOutput=False)
    lnp_d = nc.declare_dram_parameter("lnp", [C, 3], F32, isOutput=False)
    yT_d = nc.declare_dram_parameter("yT", [C, N], F32, isOutput=True)

    with tile.TileContext(nc) as tc, contextlib.ExitStack() as ctx:
        const = ctx.enter_context(tc.tile_pool(name="const", bufs=1))
        big = ctx.enter_context(tc.tile_pool(name="big", bufs=1))

        # ---- load weights + input ----
        wq_t = const.tile([128, 2, 128], F16)
        nc.sync.dma_start(out=wq_t, in_=wq_d[:, :].rearrange("(c p) m -> p c m", p=128))
        wsr_t = const.tile([128, 8, C], F16)
        nc.sync.dma_start(out=wsr_t, in_=wsr_d[:, :, :].rearrange("k p m -> p k m"))
        wk_t = const.tile([128, 2, 128], F16)
        nc.sync.dma_start(out=wk_t, in_=wk_d[:, :].rearrange("(c p) m -> p c m", p=128))
        wv_t = const.tile([128, 2, 128], F16)
        nc.sync.dma_start(out=wv_t, in_=wv_d[:, :].rearrange("(c p) m -> p c m", p=128))
        wp_t = const.tile([128, C], F32)
        nc.sync.dma_start(out=wp_t, in_=wp_d[:, :])
        lnp_t = const.tile([128, 2, 3], F32)
        nc.sync.dma_start(out=lnp_t, in_=lnp_d[:, :].rearrange("(c p) k -> p c k", p=128))
        ones16 = const.tile([128, 128], F16)
        nc.vector.memset(ones16, 1.0)
        onesC = const.tile([128, 128], F16)
        nc.vector.memset(onesC, RECIP_C)
        eps_t = const.tile([128, 1], F32)
        nc.vector.memset(eps_t, EPS)
        ec4_t = const.tile([128, 1], F32)
        nc.vector.memset(ec4_t, EXP_C4)
        zero16 = const.tile([128, QTILE], F16)
        nc.vector.memset(zero16, 0.0)
        zero32 = const.tile([128, QTILE], F32)
        nc.vector.memset(zero32, 0.0)


        # PSUM budget (8 banks): s 3x[128,2,512] = 6 (shared a/b rotation:
        # each slot's reuse lands ~1.5 iterations after its exp started, so
        # neither exp engine ever blocks an S matmul), acc 2x[128,512] = 2.
        # The pre-phase rotates through the same s slots.
        ps = ctx.enter_context(tc.tile_pool(name="ps", bufs=S_BUFS, space="PSUM"))
        acc_ps_pool = ctx.enter_context(tc.tile_pool(
            name="acc_ps", bufs=2, space="PSUM"))
        pre_sb = ctx.enter_context(tc.tile_pool(name="pre_sb", bufs=1))
        p_pool = ctx.enter_context(tc.tile_pool(name="p16", bufs=int(os.environ.get("P_BUFS", "3"))))
        w_pool = ctx.enter_context(tc.tile_pool(name="work", bufs=int(os.environ.get("W_BUFS", "3"))))

        big2 = ctx.enter_context(tc.tile_pool(name="big2", bufs=BIG2))

        env = dict(locals())
        if repeat == 1:
            _run_body(nc, tc, ctx, env)
        else:
            hints = (mybir.EngineType.PE, mybir.EngineType.Activation,
                     mybir.EngineType.DVE, mybir.EngineType.SP)
            with tc.For_i(0, repeat, 1, hint_engines=hints):
                _run_body(nc, tc, ctx, env)
    nc.compile()
    return nc


def _run_body(nc, tc, ctx, env):
    big2 = env["big2"]
    xT = big2.tile([128, 2, N], F16, tag="xT")
    qt16 = big2.tile([128, N], F16, tag="qt16")
    y16 = big2.tile([128, 2, L], F16, tag="y16")
    catn16 = big2.tile([128, 2, L], F16, tag="catn16")
    kt16 = big2.tile([128, L], F16, tag="kt16")
    v16 = big2.tile([128, NJT, 128], F16, tag="v16")
    wq_t = env["wq_t"]; wsr_t = env["wsr_t"]
    wk_t = env["wk_t"]; wv_t = env["wv_t"]; wp_t = env["wp_t"]; lnp_t = env["lnp_t"]
    ones16 = env["ones16"]; eps_t = env["eps_t"]; yT_d = env["yT_d"]
    ec4_t = env["ec4_t"]; xT_d = env["xT_d"]; onesC = env["onesC"]
    zero16 = env["zero16"]; zero32 = env["zero32"]
    ps = env["ps"]; acc_ps_pool = env["acc_ps_pool"]; pre_sb = env["pre_sb"]
    p_pool = env["p_pool"]; w_pool = env["w_pool"]

    def pre_tile():
        """[128, QTILE] f32 PSUM scratch rotating through the sa slots."""
        t = ps.tile([128, 2, QTILE], F32, tag="s", name="pre_ps")
        return t[:, 0, :]

    if PHASES == "attn":
        nc.vector.memset(qt16, 0.01)
        nc.vector.memset(kt16, 0.01)
        nc.vector.memset(v16, 0.01)
    do_pre = PHASES != "attn"
    do_attn = PHASES != "pre"

    if do_pre:
        if DMA_CHUNK:
            for cc in range(2):
                for nn in range(0, N, DMA_CHUNK):
                    nc.sync.dma_start(out=xT[:, cc, nn:nn + DMA_CHUNK],
                                      in_=xT_d[cc * 128:(cc + 1) * 128, nn:nn + DMA_CHUNK])
        else:
            nc.sync.dma_start(out=xT[:, 0, :], in_=xT_d[0:128, :])
            nc.sync.dma_start(out=xT[:, 1, :], in_=xT_d[128:256, :])

        # ---- Q^T projection (f32r): qt16[:, n] ----
        for nt in range(NQT):
            qps = pre_tile()
            for cc in range(2):
                nc.tensor.matmul(qps, wq_t[:, cc, :],
                                 xT[:, cc, nt * QTILE:(nt + 1) * QTILE],
                                 start=(cc == 0), stop=(cc == 1))
            nc.scalar.copy(qt16[:, nt * QTILE:(nt + 1) * QTILE], qps)

        # ---- strided conv (f32r): y^T [256, 1280] with bias, fp16 ----
        imgz = xT[:, :, :NZ].rearrange("p c (i j) -> p c i j", i=HZ)
        imgx = xT[:, :, NZ:].rearrange("p c (i j) -> p c i j", i=HX)
        for mt in range(2):
            for part in ('z', 0, 1):
                if part == 'z':
                    zps = pre_tile()[:, :LZ]
                    for k8 in range(8):
                        kh, kw, cc = k8 >> 2, (k8 >> 1) & 1, k8 & 1
                        rhs = imgz[:, cc, kh::2, kw::2]
                        nc.tensor.matmul(zps, wsr_t[:, k8, mt * 128:(mt + 1) * 128],
                                         rhs, start=(k8 == 0), stop=(k8 == 7))
                    nc.scalar.activation(y16[:, mt, 0:LZ], zps, AF.Identity,
                                         bias=lnp_t[:, mt, 0:1])
                else:
                    xt = part
                    xps = pre_tile()
                    for k8 in range(8):
                        kh, kw, cc = k8 >> 2, (k8 >> 1) & 1, k8 & 1
                        rhs = imgx[:, cc, 32 * xt + kh: 32 * xt + kh + 31: 2, kw::2]
                        nc.tensor.matmul(xps, wsr_t[:, k8, mt * 128:(mt + 1) * 128],
                                         rhs, start=(k8 == 0), stop=(k8 == 7))
                    nc.scalar.activation(y16[:, mt, LZ + QTILE * xt: LZ + QTILE * (xt + 1)],
                                         xps, AF.Identity, bias=lnp_t[:, mt, 0:1])

        # ---- layernorm over channels (stats via fp16 ones-matmul) ----
        ysq16 = pre_sb.tile([128, 2, L], F16, tag="ysq")
        for cc in range(2):
            nc.vector.tensor_mul(ysq16[:, cc, :], y16[:, cc, :], y16[:, cc, :])
        mean_b = pre_sb.tile([128, L], F32, tag="mean")
        var_b = pre_sb.tile([128, L], F32, tag="var")
        for off, sz in ((0, 512), (512, 512), (1024, 256)):
            s1_ps = pre_tile()
            s2_ps = pre_tile()
            for cc in range(2):
                nc.tensor.matmul(s1_ps[:, :sz], onesC,
                                 y16[:, cc, off:off + sz],
                                 start=(cc == 0), stop=(cc == 1))
                nc.tensor.matmul(s2_ps[:, :sz], onesC,
                                 ysq16[:, cc, off:off + sz],
                                 start=(cc == 0), stop=(cc == 1))
            nc.scalar.copy(mean_b[:, off:off + sz], s1_ps[:, :sz])
            nc.scalar.copy(var_b[:, off:off + sz], s2_ps[:, :sz])
        msq_b = pre_sb.tile([128, L], F32, tag="msq")
        nc.vector.tensor_mul(msq_b, mean_b, mean_b)
        std_b = pre_sb.tile([128, L], F32, tag="std")
        rstd_b = pre_sb.tile([128, L], F32, tag="rstd")
        rscr_b = pre_sb.tile([128, L], F32, tag="rscr")
        nc.vector.tensor_tensor(var_b, var_b, msq_b, OP.subtract)
        nc.scalar.activation(std_b, var_b, AF.Sqrt, bias=eps_t[:, 0:1])
        nc.vector.reciprocal_approx_accurate(rstd_b, std_b, rscr_b)
        for cc in range(2):
            t32 = pre_sb.tile([128, L], F32, tag="t32")
            nc.vector.tensor_tensor(t32, y16[:, cc, :].bitcast(F16), mean_b, OP.subtract)
            nc.vector.tensor_tensor(t32, t32, rstd_b, OP.mult)
            nc.scalar.activation(catn16[:, cc, :], t32, AF.Identity,
                                 bias=lnp_t[:, cc, 2:3],
                                 scale=lnp_t[:, cc, 1:2])

        # ---- K^T and V (fp16) ----
        for off, sz in ((0, 512), (512, 512), (1024, 256)):
            kps = pre_tile()
            for cc in range(2):
                nc.tensor.matmul(kps[:, :sz], wk_t[:, cc, :],
                                 catn16[:, cc, off:off + sz],
                                 start=(cc == 0), stop=(cc == 1))
            nc.scalar.copy(kt16[:, off:off + sz], kps[:, :sz])
        for jt0 in range(0, NJT, 4):
            nv = min(4, NJT - jt0)
            vps = ps.tile([128, 2, QTILE], F32, tag="s", name="vps")
            vflat = vps.rearrange("p c q -> p (c q)")
            for j4 in range(nv):
                jt = jt0 + j4
                for cc in range(2):
                    nc.tensor.matmul(vflat[:, j4 * 128:(j4 + 1) * 128],
                                     catn16[:, cc, jt * 128:(jt + 1) * 128],
                                     wv_t[:, cc, :],
                                     start=(cc == 0), stop=(cc == 1))
            nc.scalar.copy(
                v16[:, jt0:jt0 + nv, :].rearrange("p j m -> p (j m)"),
                vflat[:, :nv * 128])

    # ---- attention + projection, one 512-query tile at a time ----
    if do_attn:
        p16c = None
        if EXPF == "mini":
            p16c = pre_sb.tile([128, 2, QTILE], F16, tag="p16c")
            nc.gpsimd.memset(p16c, 0.00078125)

        def emit_tail(t):
            o_t, den_t, qt_t = t
            recip = w_pool.tile([128, QTILE], F32, tag="recip")
            nc.vector.reciprocal_approx_fast(recip, den_t)
            onorm = w_pool.tile([128, QTILE], F32, tag="onorm")
            nc.vector.tensor_mul(onorm, o_t, recip)
            for mt in range(2):
                if DEFER_TAIL:
                    pps = ps.tile([128, 2, QTILE], F32, tag="s", name="pps")[:, 0, :]
                else:
                    pps = acc_ps_pool.tile([128, QTILE], F32, tag="acc", name="pps")
                nc.tensor.matmul(pps, wp_t[:, mt * 128:(mt + 1) * 128], onorm,
                                 start=True, stop=True)
                ysb = w_pool.tile([128, QTILE], F32, tag="ysb")
                if PSPLIT and mt == 1:
                    nc.vector.tensor_copy(ysb, pps)
                else:
                    nc.scalar.copy(ysb, pps)
                nc.sync.dma_start(
                    out=yT_d[mt * 128:(mt + 1) * 128,
                             qt_t * QTILE:(qt_t + 1) * QTILE],
                    in_=ysb)
            if DEFER_TAIL and TAIL_DUMMY:
                ps.tile([128, 2, QTILE], F32, tag="s", name="sdummy")

        tail = None
        for qt in range(NQT):
            jts = list(range(ZJT)) if qt < ZQT else list(range(NJT))
            o_ps = acc_ps_pool.tile([128, QTILE], F32, tag="acc")
            den_ps = acc_ps_pool.tile([128, QTILE], F32, tag="acc")

            def emit_zeros():
                if MS_ZERO == "dve":
                    nc.vector.memset(o_ps, 0.0)
                    nc.vector.memset(den_ps, 0.0)
                elif MS_ZERO == "act":
                    nc.scalar.copy(o_ps, zero32)
                    nc.scalar.copy(den_ps, zero32)
                else:
                    nc.tensor.matmul(o_ps, ones16, zero16, start=True,
                                     stop=True, skip_group_check=True)
                    nc.tensor.matmul(den_ps, ones16, zero16, start=True,
                                     stop=True, skip_group_check=True)

            if not DEFER_TAIL:
                emit_zeros()
            qsl = slice(qt * QTILE, (qt + 1) * QTILE)
            prevs = []

            def emit_odens(pa, pb):
                for h in range(4):
                    p2 = pa if h < 2 else pb
                    src = p16c[:, h % 2, :] if EXPF == "mini" else p2[0][:, h % 2, :]
                    nc.tensor.matmul(o_ps[32 * h:32 * h + 32, :],
                                     v16[:, p2[1], 32 * h:32 * h + 32],
                                     src,
                                     start=False, stop=False,
                                     tile_position=(0, 32 * h),
                                     skip_group_check=True)
                if NODEN:
                    return
                for h in range(4):
                    p2 = pa if h < 2 else pb
                    src = p16c[:, h % 2, :] if EXPF == "mini" else p2[0][:, h % 2, :]
                    nc.tensor.matmul(den_ps[32 * h:32 * h + 32, :],
                                     ones16[:, 0:32], src,
                                     start=False, stop=False,
                                     tile_position=(0, 32 * h),
                                     skip_group_check=True)

            for jt in jts:
                ksl = slice(jt * 128, (jt + 1) * 128)
                sa = ps.tile([128, 2, QTILE], F32, tag="s")
                for h in range(2):
                    nc.tensor.matmul(sa[:, h, :], kt16[32 * h:32 * h + 32, ksl],
                                     qt16[32 * h:32 * h + 32, qsl],
                                     start=True, stop=True,
                                     tile_position=(32 * h, 0))
                pa = p_pool.tile([128, 2, QTILE], F16, tag="pa")
                if EXPF == "mini":
                    nc.scalar.activation(pa[:, :, 0:8], sa[:, :, 0:8],
                                         AF.Exp, scale=SCALE)
                else:
                    nc.scalar.activation(pa, sa, AF.Exp, scale=SCALE)
                if EMIT_MID and len(prevs) >= ODEN_LAG:
                    pp = prevs.pop(0)
                    emit_odens((pp[0], pp[2]), (pp[1], pp[2]))
                sb = ps.tile([128, 2, QTILE], F32, tag="s")
                for h in range(2, 4):
                    nc.tensor.matmul(sb[:, h - 2, :], kt16[32 * h:32 * h + 32, ksl],
                                     qt16[32 * h:32 * h + 32, qsl],
                                     start=True, stop=True,
                                     tile_position=(32 * h, 0))
                pb = p_pool.tile([128, 2, QTILE], F16, tag="pb")
                if EXPF == "mini":
                    nc.vector._custom_dve(EXP_OP, out=pb[:, :, 0:8], in0=sb[:, :, 0:8],
                                          in1=ec4_t, s0=EXP_C1, s1=EXP_C2,
                                          imm2=EXP_C3)
                else:
                    nc.vector._custom_dve(EXP_OP, out=pb, in0=sb,
                                          in1=ec4_t, s0=EXP_C1, s1=EXP_C2,
                                          imm2=EXP_C3)
                prevs.append((pa, pb, jt))
                if DEFER_TAIL and jt == jts[1]:
                    if tail is not None:
                        emit_tail(tail)
                    emit_zeros()
                if not EMIT_MID and len(prevs) > ODEN_LAG:
                    pp = prevs.pop(0)
                    emit_odens((pp[0], pp[2]), (pp[1], pp[2]))
            for pp in prevs:
                emit_odens((pp[0], pp[2]), (pp[1], pp[2]))

            if DEFER_TAIL:
                tail = (o_ps, den_ps, qt)
            else:
                emit_tail((o_ps, den_ps, qt))
        if DEFER_TAIL and tail is not None:
            emit_tail(tail)


def _get_nc():
    if "nc" not in _CACHED:
        _CACHED["nc"] = _build_nc()
    return _CACHED["nc"]


def _prep_inputs(x, Wq, Wkv, Wsr, bsr, gamma, beta, Wproj, bproj):
    """Build the 8 per-core input dicts (host-side shard + transpose)."""
    x = np.asarray(x, np.float32)
    Wq = np.asarray(Wq, np.float32)
    Wkv = np.asarray(Wkv, np.float32)
    Wsr = np.asarray(Wsr, np.float32)
    Wproj = np.asarray(Wproj, np.float32)
    lnp = np.ascontiguousarray(
        np.stack([np.asarray(bsr, np.float32), np.asarray(gamma, np.float32),
                  np.asarray(beta, np.float32)], axis=1))
    wsr8 = np.ascontiguousarray(
        Wsr.transpose(2, 3, 1, 0).reshape(8, 128, C).astype(np.float32))
    in_maps = []
    for core in range(NCORES):
        b, hg = core // 2, core % 2
        sl = slice(hg * 128, (hg + 1) * 128)
        in_maps.append({
            "xT": np.ascontiguousarray(x[b].T.astype(np.float16)),
            "wq": np.ascontiguousarray(Wq[sl, :].T.astype(np.float16)),
            "wsr": wsr8.astype(np.float16),
            "wk": np.ascontiguousarray(Wkv[:C][sl, :].T.astype(np.float16)),
            "wv": np.ascontiguousarray(Wkv[C:][sl, :].T.astype(np.float16)),
            "wp": np.ascontiguousarray(Wproj[:, sl].T),
            "lnp": lnp,
        })
    return in_maps


def kernel(x, Wq, Wkv, Wsr, bsr, gamma, beta, Wproj, bproj,
           H_x=64, W_x=64, H_z=32, W_z=32, _trace=False, _trace_kwargs=None):
    assert int(H_x) == HX and int(W_x) == WX and int(H_z) == HZ and int(W_z) == WZ
    nc = _get_nc()
    in_maps = _prep_inputs(x, Wq, Wkv, Wsr, bsr, gamma, beta, Wproj, bproj)
    kw = dict(_trace_kwargs or {})
    res = run_bass_kernel_spmd(nc, in_maps, core_ids=list(range(NCORES)),
                               trace=_trace, **kw)
    _CACHED["last_result"] = res
    bproj = np.asarray(bproj, np.float32)
    out = np.empty((B, N, C), np.float32)
    for b in range(B):
        yT = res.results[2 * b]["yT"] + res.results[2 * b + 1]["yT"]
        out[b] = yT.T + bproj
    return out


# revision 29
# speedup vs baseline: 1.0076x; 1.0076x over previous
"""Trainium2 Bass kernel for nn_Attention_87625922773171.

Spatial-reduction attention (PVT-style) over B=4, N=5120 (1024 template +
4096 search tokens), C=256, 8 heads, sr_ratio=2.

Sharding: 8 cores = 4 batches x 2 head-groups (4 heads each).  Each core
computes its (b, hg) slice end-to-end in a transposed ("channels on
partitions") layout; the host sums the two head-group partial projections,
adds bproj, and transposes back.

Attention loop: software-pipelined.  The S PSUM tiles rotate through a
single 3-slot pool shared by the ACT and DVE exp consumers, so each slot's
reuse lands ~1.5 iterations after its exp started and neither exp engine
ever blocks an S matmul.  O/den matmuls trail S/exp by one j-iteration and
are grouped by stationary operand so the 4 col-banded tiles run
concurrently on the PE sub-arrays.

Device dataflow per core (all layouts transposed: [channels, tokens]):
  xT [256,5120] --f32r matmul--> Q^T (fp16)
  xT --strided-AP conv matmul--> y^T --LN (stats via ones-matmul)--> cat^T (fp16)
  cat^T --fp16 matmuls--> K^T [128,1280], V [1280,128]
  per 512-query tile: S^T = K^T_h.T @ Q^T_h (row-tiled 4 heads, fp16)
      P = exp(scale*S^T)  (ACT heads 0-1, DVE poly heads 2-3, fp16)
      O += V_h.T @ P_h; den += ones.T @ P_h (col-tiled, PSUM accumulate)
      O_norm = O * recip(den);  y^T = Wproj_part.T @ O_norm (fp32)
"""
import os
import contextlib
import numpy as np

import concourse.bacc as bacc
import concourse.mybir as mybir
import concourse.tile as tile
from concourse.bass_utils import run_bass_kernel_spmd

F32 = mybir.dt.float32
F32R = mybir.dt.float32r
F16 = mybir.dt.float16
AF = mybir.ActivationFunctionType
OP = mybir.AluOpType

B, N, C = 4, 5120, 256
NHEADS, D, SR = 8, 32, 2
HZ = WZ = 32
HX = WX = 64
NZ, NX = HZ * WZ, HX * WX  # 1024, 4096
LZ, LX = (HZ // SR) * (WZ // SR), (HX // SR) * (WX // SR)  # 256, 1024
L = LZ + LX  # 1280
SCALE = float(D) ** -0.5
EPS = 1e-5
NCORES = 8
QTILE = 512
NQT = N // QTILE            # 10 query tiles (0,1 are template queries)
NJT = L // 128              # 10 key tiles (0,1 are template keys)
ZQT = NZ // QTILE           # 2
ZJT = LZ // 128             # 2

_CACHED = {}

# degree-4 polynomial exp(SCALE*s) on s in [-4.59, 4.59] (scaled logits in
# [-0.81, 0.81], measured range +2%); p(0)=1 constrained LSQ on relative
# error, max rel err 4.3e-4 at range edge.  Coefficients folded with SCALE.
_EC = (0.99932575, 0.50072616, 0.17232145, 0.04077664)
EXP_C1 = _EC[0] * SCALE
EXP_C2 = _EC[1] * SCALE ** 2
EXP_C3 = _EC[2] * SCALE ** 3
EXP_C4 = _EC[3] * SCALE ** 4


def _register_exp_op():
    import concourse.dve_ops as dvo
    from concourse.dve_spec import (
        Spec, Src0, One, C0, C1, C2, C3, _spill_c3_to_src1, _has_src1, lower)
    from concourse.dve_uop import DveOpSpec
    name = "ANT_EXP_POLY4"
    for op in dvo.OPS:
        if op.name == name:
            return op
    body = _spill_c3_to_src1(
        One + Src0 * (C0 + Src0 * (C1 + Src0 * (C2 + Src0 * C3))))

    def _ref(in0, in1, s0, s1, imm2):
        c3 = np.asarray(in1).reshape(in1.shape[0], -1)[:, :1]
        return 1.0 + in0 * (s0 + in0 * (s1 + in0 * (imm2 + in0 * c3)))

    spec = Spec(body=body, reference=_ref)
    dvo._SUB_OPCODE_FOR_NAME[name] = dvo._CUSTOM_DVE_ROW_BASE + len(dvo.OPS)
    shas = {}
    for ver in ("v3", "v4"):
        s = DveOpSpec(name=name, opcode=dvo.get_dve_sub_opcode(name),
                      uops=lower(spec, ver=ver), rd1_en=_has_src1(spec))
        shas[ver] = s.sha(ver)
    op = dvo.DveOp(name, spec, subdim=False, uops_sha=shas)
    dvo.OPS.append(op)
    dvo.CUSTOM_DVE_SPECS[name] = spec
    return op


EXP_OP = _register_exp_op()

PHASES = os.environ.get("PHASES", "all")  # all | pre | attn
MS_ZERO = os.environ.get("MS_ZERO", "act")  # dve | act | pe (acc zeroing engine)
S_BUFS = int(os.environ.get("S_BUFS", "3"))  # shared S-psum rotation slots
EXPF = os.environ.get("EXPF", "full")  # full | mini (probe: tiny exp slices)
NODEN = os.environ.get("NODEN", "0") == "1"  # probe: skip den matmuls
BIG2 = int(os.environ.get("BIG2", "2"))  # big-tensor bufs (2 = cross-body dbuf)
PSPLIT = os.environ.get("PSPLIT", "1") == "1"  # proj evac on ACT+DVE
DMA_CHUNK = int(os.environ.get("DMA_CHUNK", "2560"))  # xT DMA chunk (0 = off)
ODEN_LAG = int(os.environ.get("ODEN_LAG", "2"))  # O/den trail distance (jts)
DEFER_TAIL = os.environ.get("DEFER_TAIL", "1") == "1"  # qt tail into next qt
TAIL_DUMMY = os.environ.get("TAIL_DUMMY", "0") == "1"  # keep s-slot parity
EMIT_MID = os.environ.get("EMIT_MID", "0") == "1"  # O/den between S_a and S_b
LAG_B = int(os.environ.get("LAG_B", "2"))  # DVE-pair O/den trail distance
RECIP_C = 1.0 / C


def _build_nc(repeat=1):
    nc = bacc.Bacc("TRN2", target_bir_lowering=False)

    xT_d = nc.declare_dram_parameter("xT", [C, N], F16, isOutput=False)
    wq_d = nc.declare_dram_parameter("wq", [C, 128], F16, isOutput=False)
    wsr_d = nc.declare_dram_parameter("wsr", [8, 128, C], F16, isOutput=False)
    wk_d = nc.declare_dram_parameter("wk", [C, 128], F16, isOutput=False)
    wv_d = nc.declare_dram_parameter("wv", [C, 128], F16, isOutput=False)
    wp_d = nc.declare_dram_parameter("wp", [128, C], F32, isOutput=False)
    lnp_d = nc.declare_dram_parameter("lnp", [C, 3], F32, isOutput=False)
    yT_d = nc.declare_dram_parameter("yT", [C, N], F32, isOutput=True)

    with tile.TileContext(nc) as tc, contextlib.ExitStack() as ctx:
        const = ctx.enter_context(tc.tile_pool(name="const", bufs=1))
        big = ctx.enter_context(tc.tile_pool(name="big", bufs=1))

        # ---- load weights + input ----
        wq_t = const.tile([128, 2, 128], F16)
        nc.sync.dma_start(out=wq_t, in_=wq_d[:, :].rearrange("(c p) m -> p c m", p=128))
        wsr_t = const.tile([128, 8, C], F16)
        nc.sync.dma_start(out=wsr_t, in_=wsr_d[:, :, :].rearrange("k p m -> p k m"))
        wk_t = const.tile([128, 2, 128], F16)
        nc.sync.dma_start(out=wk_t, in_=wk_d[:, :].rearrange("(c p) m -> p c m", p=128))
        wv_t = const.tile([128, 2, 128], F16)
        nc.sync.dma_start(out=wv_t, in_=wv_d[:, :].rearrange("(c p) m -> p c m", p=128))
        wp_t = const.tile([128, C], F32)
        nc.sync.dma_start(out=wp_t, in_=wp_d[:, :])
        lnp_t = const.tile([128, 2, 3], F32)
        nc.sync.dma_start(out=lnp_t, in_=lnp_d[:, :].rearrange("(c p) k -> p c k", p=128))
        ones16 = const.tile([128, 128], F16)
        nc.vector.memset(ones16, 1.0)
        onesC = const.tile([128, 128], F16)
        nc.vector.memset(onesC, RECIP_C)
        eps_t = const.tile([128, 1], F32)
        nc.vector.memset(eps_t, EPS)
        ec4_t = const.tile([128, 1], F32)
        nc.vector.memset(ec4_t, EXP_C4)
        zero16 = const.tile([128, QTILE], F16)
        nc.vector.memset(zero16, 0.0)
        zero32 = const.tile([128, QTILE], F32)
        nc.vector.memset(zero32, 0.0)


        # PSUM budget (8 banks): s 3x[128,2,512] = 6 (shared a/b rotation:
        # each slot's reuse lands ~1.5 iterations after its exp started, so
        # neither exp engine ever blocks an S matmul), acc 2x[128,512] = 2.
        # The pre-phase rotates through the same s slots.
        ps = ctx.enter_context(tc.tile_pool(name="ps", bufs=S_BUFS, space="PSUM"))
        acc_ps_pool = ctx.enter_context(tc.tile_pool(
            name="acc_ps", bufs=2, space="PSUM"))
        pre_sb = ctx.enter_context(tc.tile_pool(name="pre_sb", bufs=1))
        p_pool = ctx.enter_context(tc.tile_pool(name="p16", bufs=int(os.environ.get("P_BUFS", "3"))))
        w_pool = ctx.enter_context(tc.tile_pool(name="work", bufs=int(os.environ.get("W_BUFS", "3"))))

        big2 = ctx.enter_context(tc.tile_pool(name="big2", bufs=BIG2))

        env = dict(locals())
        if repeat == 1:
            _run_body(nc, tc, ctx, env)
        else:
            hints = (mybir.EngineType.PE, mybir.EngineType.Activation,
                     mybir.EngineType.DVE, mybir.EngineType.SP)
            with tc.For_i(0, repeat, 1, hint_engines=hints):
                _run_body(nc, tc, ctx, env)
    nc.compile()
    return nc


def _run_body(nc, tc, ctx, env):
    big2 = env["big2"]
    xT = big2.tile([128, 2, N], F16, tag="xT")
    qt16 = big2.tile([128, N], F16, tag="qt16")
    y16 = big2.tile([128, 2, L], F16, tag="y16")
    catn16 = big2.tile([128, 2, L], F16, tag="catn16")
    kt16 = big2.tile([128, L], F16, tag="kt16")
    v16 = big2.tile([128, NJT, 128], F16, tag="v16")
    wq_t = env["wq_t"]; wsr_t = env["wsr_t"]
    wk_t = env["wk_t"]; wv_t = env["wv_t"]; wp_t = env["wp_t"]; lnp_t = env["lnp_t"]
    ones16 = env["ones16"]; eps_t = env["eps_t"]; yT_d = env["yT_d"]
    ec4_t = env["ec4_t"]; xT_d = env["xT_d"]; onesC = env["onesC"]
    zero16 = env["zero16"]; zero32 = env["zero32"]
    ps = env["ps"]; acc_ps_pool = env["acc_ps_pool"]; pre_sb = env["pre_sb"]
    p_pool = env["p_pool"]; w_pool = env["w_pool"]

    def pre_tile():
        """[128, QTILE] f32 PSUM scratch rotating through the sa slots."""
        t = ps.tile([128, 2, QTILE], F32, tag="s", name="pre_ps")
        return t[:, 0, :]

    if PHASES == "attn":
        nc.vector.memset(qt16, 0.01)
        nc.vector.memset(kt16, 0.01)
        nc.vector.memset(v16, 0.01)
    do_pre = PHASES != "attn"
    do_attn = PHASES != "pre"

    if do_pre:
        if DMA_CHUNK:
            for cc in range(2):
                for nn in range(0, N, DMA_CHUNK):
                    nc.sync.dma_start(out=xT[:, cc, nn:nn + DMA_CHUNK],
                                      in_=xT_d[cc * 128:(cc + 1) * 128, nn:nn + DMA_CHUNK])
        else:
            nc.sync.dma_start(out=xT[:, 0, :], in_=xT_d[0:128, :])
            nc.sync.dma_start(out=xT[:, 1, :], in_=xT_d[128:256, :])

        # ---- Q^T projection (f32r): qt16[:, n] ----
        for nt in range(NQT):
            qps = pre_tile()
            for cc in range(2):
                nc.tensor.matmul(qps, wq_t[:, cc, :],
                                 xT[:, cc, nt * QTILE:(nt + 1) * QTILE],
                                 start=(cc == 0), stop=(cc == 1))
            nc.scalar.copy(qt16[:, nt * QTILE:(nt + 1) * QTILE], qps)

        # ---- strided conv (f32r): y^T [256, 1280] with bias, fp16 ----
        imgz = xT[:, :, :NZ].rearrange("p c (i j) -> p c i j", i=HZ)
        imgx = xT[:, :, NZ:].rearrange("p c (i j) -> p c i j", i=HX)
        for mt in range(2):
            for part in ('z', 0, 1):
                if part == 'z':
                    zps = pre_tile()[:, :LZ]
                    for k8 in range(8):
                        kh, kw, cc = k8 >> 2, (k8 >> 1) & 1, k8 & 1
                        rhs = imgz[:, cc, kh::2, kw::2]
                        nc.tensor.matmul(zps, wsr_t[:, k8, mt * 128:(mt + 1) * 128],
                                         rhs, start=(k8 == 0), stop=(k8 == 7))
                    nc.scalar.activation(y16[:, mt, 0:LZ], zps, AF.Identity,
                                         bias=lnp_t[:, mt, 0:1])
                else:
                    xt = part
                    xps = pre_tile()
                    for k8 in range(8):
                        kh, kw, cc = k8 >> 2, (k8 >> 1) & 1, k8 & 1
                        rhs = imgx[:, cc, 32 * xt + kh: 32 * xt + kh + 31: 2, kw::2]
                        nc.tensor.matmul(xps, wsr_t[:, k8, mt * 128:(mt + 1) * 128],
                                         rhs, start=(k8 == 0), stop=(k8 == 7))
                    nc.scalar.activation(y16[:, mt, LZ + QTILE * xt: LZ + QTILE * (xt + 1)],
                                         xps, AF.Identity, bias=lnp_t[:, mt, 0:1])

        # ---- layernorm over channels (stats via fp16 ones-matmul) ----
        ysq16 = pre_sb.tile([128, 2, L], F16, tag="ysq")
        for cc in range(2):
            nc.vector.tensor_mul(ysq16[:, cc, :], y16[:, cc, :], y16[:, cc, :])
        mean_b = pre_sb.tile([128, L], F32, tag="mean")
        var_b = pre_sb.tile([128, L], F32, tag="var")
        for off, sz in ((0, 512), (512, 512), (1024, 256)):
            s1_ps = pre_tile()
            s2_ps = pre_tile()
            for cc in range(2):
                nc.tensor.matmul(s1_ps[:, :sz], onesC,
                                 y16[:, cc, off:off + sz],
                                 start=(cc == 0), stop=(cc == 1))
                nc.tensor.matmul(s2_ps[:, :sz], onesC,
                                 ysq16[:, cc, off:off + sz],
                                 start=(cc == 0), stop=(cc == 1))
            nc.scalar.copy(mean_b[:, off:off + sz], s1_ps[:, :sz])
            nc.scalar.copy(var_b[:, off:off + sz], s2_ps[:, :sz])
        msq_b = pre_sb.tile([128, L], F32, tag="msq")
        nc.vector.tensor_mul(msq_b, mean_b, mean_b)
        std_b = pre_sb.tile([128, L], F32, tag="std")
        rstd_b = pre_sb.tile([128, L], F32, tag="rstd")
        rscr_b = pre_sb.tile([128, L], F32, tag="rscr")
        nc.vector.tensor_tensor(var_b, var_b, msq_b, OP.subtract)
        nc.scalar.activation(std_b, var_b, AF.Sqrt, bias=eps_t[:, 0:1])
        nc.vector.reciprocal_approx_accurate(rstd_b, std_b, rscr_b)
        for cc in range(2):
            t32 = pre_sb.tile([128, L], F32, tag="t32")
            nc.vector.tensor_tensor(t32, y16[:, cc, :].bitcast(F16), mean_b, OP.subtract)
            nc.vector.tensor_tensor(t32, t32, rstd_b, OP.mult)
            nc.scalar.activation(catn16[:, cc, :], t32, AF.Identity,
                                 bias=lnp_t[:, cc, 2:3],
                                 scale=lnp_t[:, cc, 1:2])

        # ---- K^T and V (fp16) ----
        for off, sz in ((0, 512), (512, 512), (1024, 256)):
            kps = pre_tile()
            for cc in range(2):
                nc.tensor.matmul(kps[:, :sz], wk_t[:, cc, :],
                                 catn16[:, cc, off:off + sz],
                                 start=(cc == 0), stop=(cc == 1))
            nc.scalar.copy(kt16[:, off:off + sz], kps[:, :sz])
        for jt0 in range(0, NJT, 4):
            nv = min(4, NJT - jt0)
            vps = ps.tile([128, 2, QTILE], F32, tag="s", name="vps")
            vflat = vps.rearrange("p c q -> p (c q)")
            for j4 in range(nv):
                jt = jt0 + j4
                for cc in range(2):
                    nc.tensor.matmul(vflat[:, j4 * 128:(j4 + 1) * 128],
                                     catn16[:, cc, jt * 128:(jt + 1) * 128],
                                     wv_t[:, cc, :],
                                     start=(cc == 0), stop=(cc == 1))
            nc.scalar.copy(
                v16[:, jt0:jt0 + nv, :].rearrange("p j m -> p (j m)"),
                vflat[:, :nv * 128])

    # ---- attention + projection, one 512-query tile at a time ----
    if do_attn:
        p16c = None
        if EXPF == "mini":
            p16c = pre_sb.tile([128, 2, QTILE], F16, tag="p16c")
            nc.gpsimd.memset(p16c, 0.00078125)

        def emit_tail(t):
            o_t, den_t, qt_t = t
            recip = w_pool.tile([128, QTILE], F32, tag="recip")
            nc.vector.reciprocal_approx_fast(recip, den_t)
            onorm = w_pool.tile([128, QTILE], F32, tag="onorm")
            nc.vector.tensor_mul(onorm, o_t, recip)
            for mt in range(2):
                if DEFER_TAIL:
                    pps = ps.tile([128, 2, QTILE], F32, tag="s", name="pps")[:, 0, :]
                else:
                    pps = acc_ps_pool.tile([128, QTILE], F32, tag="acc", name="pps")
                nc.tensor.matmul(pps, wp_t[:, mt * 128:(mt + 1) * 128], onorm,
                                 start=True, stop=True)
                ysb = w_pool.tile([128, QTILE], F32, tag="ysb")
                if PSPLIT and mt == 1:
                    nc.vector.tensor_copy(ysb, pps)
                else:
                    nc.scalar.copy(ysb, pps)
                nc.sync.dma_start(
                    out=yT_d[mt * 128:(mt + 1) * 128,
                             qt_t * QTILE:(qt_t + 1) * QTILE],
                    in_=ysb)
            if DEFER_TAIL and TAIL_DUMMY:
                ps.tile([128, 2, QTILE], F32, tag="s", name="sdummy")

        tail = None
        for qt in range(NQT):
            jts = list(range(ZJT)) if qt < ZQT else list(range(NJT))
            o_ps = acc_ps_pool.tile([128, QTILE], F32, tag="acc")
            den_ps = acc_ps_pool.tile([128, QTILE], F32, tag="acc")

            def emit_zeros():
                if MS_ZERO == "dve":
                    nc.vector.memset(o_ps, 0.0)
                    nc.vector.memset(den_ps, 0.0)
                elif MS_ZERO == "act":
                    nc.scalar.copy(o_ps, zero32)
                    nc.scalar.copy(den_ps, zero32)
                else:
                    nc.tensor.matmul(o_ps, ones16, zero16, start=True,
                                     stop=True, skip_group_check=True)
                    nc.tensor.matmul(den_ps, ones16, zero16, start=True,
                                     stop=True, skip_group_check=True)

            if not DEFER_TAIL:
                emit_zeros()
            qsl = slice(qt * QTILE, (qt + 1) * QTILE)
            prevs_a, prevs_b = [], []

            def emit_pair(p2, jt2, h0):
                for hh in range(2):
                    h = h0 + hh
                    src = p16c[:, hh, :] if EXPF == "mini" else p2[:, hh, :]
                    nc.tensor.matmul(o_ps[32 * h:32 * h + 32, :],
                                     v16[:, jt2, 32 * h:32 * h + 32],
                                     src,
                                     start=False, stop=False,
                                     tile_position=(0, 32 * h),
                                     skip_group_check=True)
                if NODEN:
                    return
                for hh in range(2):
                    h = h0 + hh
                    src = p16c[:, hh, :] if EXPF == "mini" else p2[:, hh, :]
                    nc.tensor.matmul(den_ps[32 * h:32 * h + 32, :],
                                     ones16[:, 0:32], src,
                                     start=False, stop=False,
                                     tile_position=(0, 32 * h),
                                     skip_group_check=True)

            def emit_odens(pa, pb):
                for h in range(4):
                    pp = pa if h < 2 else pb
                    src = (p16c[:, h % 2, :] if EXPF == "mini"
                           else pp[0][:, h % 2, :])
                    nc.tensor.matmul(o_ps[32 * h:32 * h + 32, :],
                                     v16[:, pp[1], 32 * h:32 * h + 32],
                                     src, start=False, stop=False,
                                     tile_position=(0, 32 * h),
                                     skip_group_check=True)
                if NODEN:
                    return
                for h in range(4):
                    pp = pa if h < 2 else pb
                    src = (p16c[:, h % 2, :] if EXPF == "mini"
                           else pp[0][:, h % 2, :])
                    nc.tensor.matmul(den_ps[32 * h:32 * h + 32, :],
                                     ones16[:, 0:32], src,
                                     start=False, stop=False,
                                     tile_position=(0, 32 * h),
                                     skip_group_check=True)

            for jt in jts:
                ksl = slice(jt * 128, (jt + 1) * 128)
                sa = ps.tile([128, 2, QTILE], F32, tag="s")
                for h in range(2):
                    nc.tensor.matmul(sa[:, h, :], kt16[32 * h:32 * h + 32, ksl],
                                     qt16[32 * h:32 * h + 32, qsl],
                                     start=True, stop=True,
                                     tile_position=(32 * h, 0))
                pa = p_pool.tile([128, 2, QTILE], F16, tag="pa")
                if EXPF == "mini":
                    nc.scalar.activation(pa[:, :, 0:8], sa[:, :, 0:8],
                                         AF.Exp, scale=SCALE)
                else:
                    nc.scalar.activation(pa, sa, AF.Exp, scale=SCALE)
                if EMIT_MID and len(prevs_a) >= ODEN_LAG:
                    pp = prevs_a.pop(0)
                    emit_pair(pp[0], pp[1], 0)
                    if prevs_b:
                        pp = prevs_b.pop(0)
                        emit_pair(pp[0], pp[1], 2)
                sb = ps.tile([128, 2, QTILE], F32, tag="s")
                for h in range(2, 4):
                    nc.tensor.matmul(sb[:, h - 2, :], kt16[32 * h:32 * h + 32, ksl],
                                     qt16[32 * h:32 * h + 32, qsl],
                                     start=True, stop=True,
                                     tile_position=(32 * h, 0))
                pb = p_pool.tile([128, 2, QTILE], F16, tag="pb", bufs=LAG_B + 2)
                if EXPF == "mini":
                    nc.vector._custom_dve(EXP_OP, out=pb[:, :, 0:8], in0=sb[:, :, 0:8],
                                          in1=ec4_t, s0=EXP_C1, s1=EXP_C2,
                                          imm2=EXP_C3)
                else:
                    nc.vector._custom_dve(EXP_OP, out=pb, in0=sb,
                                          in1=ec4_t, s0=EXP_C1, s1=EXP_C2,
                                          imm2=EXP_C3)
                prevs_a.append((pa, jt))
                prevs_b.append((pb, jt))
                if DEFER_TAIL and jt == jts[1]:
                    if tail is not None:
                        emit_tail(tail)
                    emit_zeros()
                if not EMIT_MID:
                    if LAG_B == ODEN_LAG:
                        if len(prevs_a) > ODEN_LAG:
                            pa2 = prevs_a.pop(0)
                            pb2 = prevs_b.pop(0)
                            emit_odens(pa2, pb2)
                    else:
                        if len(prevs_a) > ODEN_LAG:
                            pp = prevs_a.pop(0)
                            emit_pair(pp[0], pp[1], 0)
                        if len(prevs_b) > LAG_B:
                            pp = prevs_b.pop(0)
                            emit_pair(pp[0], pp[1], 2)
            if LAG_B == ODEN_LAG:
                for pa2, pb2 in zip(prevs_a, prevs_b):
                    emit_odens(pa2, pb2)
            else:
                for pp in prevs_a:
                    emit_pair(pp[0], pp[1], 0)
                for pp in prevs_b:
                    emit_pair(pp[0], pp[1], 2)

            if DEFER_TAIL:
                tail = (o_ps, den_ps, qt)
            else:
                emit_tail((o_ps, den_ps, qt))
        if DEFER_TAIL and tail is not None:
            emit_tail(tail)


def _get_nc():
    if "nc" not in _CACHED:
        _CACHED["nc"] = _build_nc()
    return _CACHED["nc"]


def _prep_inputs(x, Wq, Wkv, Wsr, bsr, gamma, beta, Wproj, bproj):
    """Build the 8 per-core input dicts (host-side shard + transpose)."""
    x = np.asarray(x, np.float32)
    Wq = np.asarray(Wq, np.float32)
    Wkv = np.asarray(Wkv, np.float32)
    Wsr = np.asarray(Wsr, np.float32)
    Wproj = np.asarray(Wproj, np.float32)
    lnp = np.ascontiguousarray(
        np.stack([np.asarray(bsr, np.float32), np.asarray(gamma, np.float32),
                  np.asarray(beta, np.float32)], axis=1))
    wsr8 = np.ascontiguousarray(
        Wsr.transpose(2, 3, 1, 0).reshape(8, 128, C).astype(np.float32))
    in_maps = []
    for core in range(NCORES):
        b, hg = core // 2, core % 2
        sl = slice(hg * 128, (hg + 1) * 128)
        in_maps.append({
            "xT": np.ascontiguousarray(x[b].T.astype(np.float16)),
            "wq": np.ascontiguousarray(Wq[sl, :].T.astype(np.float16)),
            "wsr": wsr8.astype(np.float16),
            "wk": np.ascontiguousarray(Wkv[:C][sl, :].T.astype(np.float16)),
            "wv": np.ascontiguousarray(Wkv[C:][sl, :].T.astype(np.float16)),
            "wp": np.ascontiguousarray(Wproj[:, sl].T),
            "lnp": lnp,
        })
    return in_maps


def kernel(x, Wq, Wkv, Wsr, bsr, gamma, beta, Wproj, bproj,
           H_x=64, W_x=64, H_z=32, W_z=32, _trace=False, _trace_kwargs=None):
    assert int(H_x) == HX and int(W_x) == WX and int(H_z) == HZ and int(W_z) == WZ
    nc = _get_nc()
    in_maps = _prep_inputs(x, Wq, Wkv, Wsr, bsr, gamma, beta, Wproj, bproj)
    kw = dict(_trace_kwargs or {})
    res = run_bass_kernel_spmd(nc, in_maps, core_ids=list(range(NCORES)),
                               trace=_trace, **kw)
    _CACHED["last_result"] = res
    bproj = np.asarray(bproj, np.float32)
    out = np.empty((B, N, C), np.float32)
    for b in range(B):
        yT = res.results[2 * b]["yT"] + res.results[2 * b + 1]["yT"]
        out[b] = yT.T + bproj
    return out


# revision 30
# speedup vs baseline: 1.0603x; 1.0523x over previous
"""Trainium2 Bass kernel for nn_Attention_87625922773171.

Spatial-reduction attention (PVT-style) over B=4, N=5120 (1024 template +
4096 search tokens), C=256, 8 heads, sr_ratio=2.

Sharding: 8 cores = 4 batches x 2 head-groups (4 heads each).  Each core
computes its (b, hg) slice end-to-end in a transposed ("channels on
partitions") layout; the host sums the two head-group partial projections,
adds bproj, and transposes back.

Attention loop: software-pipelined.  The S PSUM tiles rotate through a
single 3-slot pool shared by the ACT and DVE exp consumers, so each slot's
reuse lands ~1.5 iterations after its exp started and neither exp engine
ever blocks an S matmul.  O/den matmuls trail S/exp by one j-iteration and
are grouped by stationary operand so the 4 col-banded tiles run
concurrently on the PE sub-arrays.

Device dataflow per core (all layouts transposed: [channels, tokens]):
  xT [256,5120] --f32r matmul--> Q^T (fp16)
  xT --strided-AP conv matmul--> y^T --LN (stats via ones-matmul)--> cat^T (fp16)
  cat^T --fp16 matmuls--> K^T [128,1280], V [1280,128]
  per 512-query tile: S^T = K^T_h.T @ Q^T_h (row-tiled 4 heads, fp16)
      P = exp(scale*S^T)  (ACT heads 0-1, DVE poly heads 2-3, fp16)
      O += V_h.T @ P_h; den += ones.T @ P_h (col-tiled, PSUM accumulate)
      O_norm = O * recip(den);  y^T = Wproj_part.T @ O_norm (fp32)
"""
import os
import contextlib
import numpy as np

import concourse.bacc as bacc
import concourse.mybir as mybir
import concourse.tile as tile
from concourse.bass_utils import run_bass_kernel_spmd

F32 = mybir.dt.float32
F32R = mybir.dt.float32r
F16 = mybir.dt.float16
AF = mybir.ActivationFunctionType
OP = mybir.AluOpType

B, N, C = 4, 5120, 256
NHEADS, D, SR = 8, 32, 2
HZ = WZ = 32
HX = WX = 64
NZ, NX = HZ * WZ, HX * WX  # 1024, 4096
LZ, LX = (HZ // SR) * (WZ // SR), (HX // SR) * (WX // SR)  # 256, 1024
L = LZ + LX  # 1280
SCALE = float(D) ** -0.5
EPS = 1e-5
NCORES = 8
QTILE = 512
NQT = N // QTILE            # 10 query tiles (0,1 are template queries)
NJT = L // 128              # 10 key tiles (0,1 are template keys)
ZQT = NZ // QTILE           # 2
ZJT = LZ // 128             # 2

_CACHED = {}

# degree-4 polynomial exp(SCALE*s) on s in [-4.59, 4.59] (scaled logits in
# [-0.81, 0.81], measured range +2%); p(0)=1 constrained LSQ on relative
# error, max rel err 4.3e-4 at range edge.  Coefficients folded with SCALE.
_EC = (0.99932575, 0.50072616, 0.17232145, 0.04077664)
EXP_C1 = _EC[0] * SCALE
EXP_C2 = _EC[1] * SCALE ** 2
EXP_C3 = _EC[2] * SCALE ** 3
EXP_C4 = _EC[3] * SCALE ** 4


def _register_exp_op():
    import concourse.dve_ops as dvo
    from concourse.dve_spec import (
        Spec, Src0, One, C0, C1, C2, C3, _spill_c3_to_src1, _has_src1, lower)
    from concourse.dve_uop import DveOpSpec
    name = "ANT_EXP_POLY4"
    for op in dvo.OPS:
        if op.name == name:
            return op
    body = _spill_c3_to_src1(
        One + Src0 * (C0 + Src0 * (C1 + Src0 * (C2 + Src0 * C3))))

    def _ref(in0, in1, s0, s1, imm2):
        c3 = np.asarray(in1).reshape(in1.shape[0], -1)[:, :1]
        return 1.0 + in0 * (s0 + in0 * (s1 + in0 * (imm2 + in0 * c3)))

    spec = Spec(body=body, reference=_ref)
    dvo._SUB_OPCODE_FOR_NAME[name] = dvo._CUSTOM_DVE_ROW_BASE + len(dvo.OPS)
    shas = {}
    for ver in ("v3", "v4"):
        s = DveOpSpec(name=name, opcode=dvo.get_dve_sub_opcode(name),
                      uops=lower(spec, ver=ver), rd1_en=_has_src1(spec))
        shas[ver] = s.sha(ver)
    op = dvo.DveOp(name, spec, subdim=False, uops_sha=shas)
    dvo.OPS.append(op)
    dvo.CUSTOM_DVE_SPECS[name] = spec
    return op


EXP_OP = _register_exp_op()

PHASES = os.environ.get("PHASES", "all")  # all | pre | attn
MS_ZERO = os.environ.get("MS_ZERO", "act")  # dve | act | pe (acc zeroing engine)
S_BUFS = int(os.environ.get("S_BUFS", "3"))  # shared S-psum rotation slots
EXPF = os.environ.get("EXPF", "full")  # full | mini (probe: tiny exp slices)
NODEN = os.environ.get("NODEN", "0") == "1"  # probe: skip den matmuls
BIG2 = int(os.environ.get("BIG2", "2"))  # big-tensor bufs (2 = cross-body dbuf)
PSPLIT = os.environ.get("PSPLIT", "1") == "1"  # proj evac on ACT+DVE
DMA_CHUNK = int(os.environ.get("DMA_CHUNK", "2560"))  # xT DMA chunk (0 = off)
ODEN_LAG = int(os.environ.get("ODEN_LAG", "2"))  # O/den trail distance (jts)
DEFER_TAIL = os.environ.get("DEFER_TAIL", "1") == "1"  # qt tail into next qt
TAIL_DUMMY = os.environ.get("TAIL_DUMMY", "0") == "1"  # keep s-slot parity
EMIT_MID = os.environ.get("EMIT_MID", "0") == "1"  # O/den between S_a and S_b
LAG_B = int(os.environ.get("LAG_B", "2"))  # DVE-pair O/den trail distance
PRE_SPLIT = os.environ.get("PRE_SPLIT", "0") == "1"  # pre evacs on ACT+DVE
RECIP_C = 1.0 / C


def _build_nc(repeat=1):
    nc = bacc.Bacc("TRN2", target_bir_lowering=False)

    xT_d = nc.declare_dram_parameter("xT", [C, N], F16, isOutput=False)
    wq_d = nc.declare_dram_parameter("wq", [C, 128], F16, isOutput=False)
    wsr_d = nc.declare_dram_parameter("wsr", [8, 128, C], F16, isOutput=False)
    wk_d = nc.declare_dram_parameter("wk", [C, 128], F16, isOutput=False)
    wv_d = nc.declare_dram_parameter("wv", [C, 128], F16, isOutput=False)
    wp_d = nc.declare_dram_parameter("wp", [128, C], F32, isOutput=False)
    lnp_d = nc.declare_dram_parameter("lnp", [C, 3], F32, isOutput=False)
    yT_d = nc.declare_dram_parameter("yT", [C, N], F32, isOutput=True)

    with tile.TileContext(nc) as tc, contextlib.ExitStack() as ctx:
        const = ctx.enter_context(tc.tile_pool(name="const", bufs=1))
        big = ctx.enter_context(tc.tile_pool(name="big", bufs=1))

        # ---- load weights + input ----
        wq_t = const.tile([128, 2, 128], F16)
        nc.sync.dma_start(out=wq_t, in_=wq_d[:, :].rearrange("(c p) m -> p c m", p=128))
        wsr_t = const.tile([128, 8, C], F16)
        nc.sync.dma_start(out=wsr_t, in_=wsr_d[:, :, :].rearrange("k p m -> p k m"))
        wk_t = const.tile([128, 2, 128], F16)
        nc.sync.dma_start(out=wk_t, in_=wk_d[:, :].rearrange("(c p) m -> p c m", p=128))
        wv_t = const.tile([128, 2, 128], F16)
        nc.sync.dma_start(out=wv_t, in_=wv_d[:, :].rearrange("(c p) m -> p c m", p=128))
        wp_t = const.tile([128, C], F32)
        nc.sync.dma_start(out=wp_t, in_=wp_d[:, :])
        lnp_t = const.tile([128, 2, 3], F32)
        nc.sync.dma_start(out=lnp_t, in_=lnp_d[:, :].rearrange("(c p) k -> p c k", p=128))
        ones16 = const.tile([128, 128], F16)
        nc.vector.memset(ones16, 1.0)
        onesC = const.tile([128, 128], F16)
        nc.vector.memset(onesC, RECIP_C)
        eps_t = const.tile([128, 1], F32)
        nc.vector.memset(eps_t, EPS)
        ec4_t = const.tile([128, 1], F32)
        nc.vector.memset(ec4_t, EXP_C4)
        zero16 = const.tile([128, QTILE], F16)
        nc.vector.memset(zero16, 0.0)
        zero32 = const.tile([128, QTILE], F32)
        nc.vector.memset(zero32, 0.0)


        # PSUM budget (8 banks): s 3x[128,2,512] = 6 (shared a/b rotation:
        # each slot's reuse lands ~1.5 iterations after its exp started, so
        # neither exp engine ever blocks an S matmul), acc 2x[128,512] = 2.
        # The pre-phase rotates through the same s slots.
        ps = ctx.enter_context(tc.tile_pool(name="ps", bufs=S_BUFS, space="PSUM"))
        acc_ps_pool = ctx.enter_context(tc.tile_pool(
            name="acc_ps", bufs=2, space="PSUM"))
        pre_sb = ctx.enter_context(tc.tile_pool(name="pre_sb", bufs=1))
        p_pool = ctx.enter_context(tc.tile_pool(name="p16", bufs=int(os.environ.get("P_BUFS", "3"))))
        w_pool = ctx.enter_context(tc.tile_pool(name="work", bufs=int(os.environ.get("W_BUFS", "3"))))

        big2 = ctx.enter_context(tc.tile_pool(name="big2", bufs=BIG2))

        env = dict(locals())
        if repeat == 1:
            _run_body(nc, tc, ctx, env)
        else:
            hints = (mybir.EngineType.PE, mybir.EngineType.Activation,
                     mybir.EngineType.DVE, mybir.EngineType.SP)
            with tc.For_i(0, repeat, 1, hint_engines=hints):
                _run_body(nc, tc, ctx, env)
    nc.compile()
    return nc


def _run_body(nc, tc, ctx, env):
    big2 = env["big2"]
    xT = big2.tile([128, 2, N], F16, tag="xT")
    qt16 = big2.tile([128, N], F16, tag="qt16")
    y16 = big2.tile([128, 2, L], F16, tag="y16")
    catn16 = big2.tile([128, 2, L], F16, tag="catn16")
    kt16 = big2.tile([128, L], F16, tag="kt16")
    v16 = big2.tile([128, NJT, 128], F16, tag="v16")
    wq_t = env["wq_t"]; wsr_t = env["wsr_t"]
    wk_t = env["wk_t"]; wv_t = env["wv_t"]; wp_t = env["wp_t"]; lnp_t = env["lnp_t"]
    ones16 = env["ones16"]; eps_t = env["eps_t"]; yT_d = env["yT_d"]
    ec4_t = env["ec4_t"]; xT_d = env["xT_d"]; onesC = env["onesC"]
    zero16 = env["zero16"]; zero32 = env["zero32"]
    ps = env["ps"]; acc_ps_pool = env["acc_ps_pool"]; pre_sb = env["pre_sb"]
    p_pool = env["p_pool"]; w_pool = env["w_pool"]

    def pre_tile():
        """[128, QTILE] f32 PSUM scratch rotating through the sa slots."""
        t = ps.tile([128, 2, QTILE], F32, tag="s", name="pre_ps")
        return t[:, 0, :]

    if PHASES == "attn":
        nc.vector.memset(qt16, 0.01)
        nc.vector.memset(kt16, 0.01)
        nc.vector.memset(v16, 0.01)
    do_pre = PHASES != "attn"
    do_attn = PHASES != "pre"

    if do_pre:
        if DMA_CHUNK:
            for cc in range(2):
                for nn in range(0, N, DMA_CHUNK):
                    nc.sync.dma_start(out=xT[:, cc, nn:nn + DMA_CHUNK],
                                      in_=xT_d[cc * 128:(cc + 1) * 128, nn:nn + DMA_CHUNK])
        else:
            nc.sync.dma_start(out=xT[:, 0, :], in_=xT_d[0:128, :])
            nc.sync.dma_start(out=xT[:, 1, :], in_=xT_d[128:256, :])

        # ---- Q^T projection (f32r): qt16[:, n] ----
        for nt in range(NQT):
            qps = pre_tile()
            for cc in range(2):
                nc.tensor.matmul(qps, wq_t[:, cc, :],
                                 xT[:, cc, nt * QTILE:(nt + 1) * QTILE],
                                 start=(cc == 0), stop=(cc == 1))
            if PRE_SPLIT and nt % 2 == 1:
                nc.vector.tensor_copy(qt16[:, nt * QTILE:(nt + 1) * QTILE], qps)
            else:
                nc.scalar.copy(qt16[:, nt * QTILE:(nt + 1) * QTILE], qps)

        # ---- strided conv (f32r): y^T [256, 1280] with bias, fp16 ----
        imgz = xT[:, :, :NZ].rearrange("p c (i j) -> p c i j", i=HZ)
        imgx = xT[:, :, NZ:].rearrange("p c (i j) -> p c i j", i=HX)
        for mt in range(2):
            for part in ('z', 0, 1):
                if part == 'z':
                    zps = pre_tile()[:, :LZ]
                    for k8 in range(8):
                        kh, kw, cc = k8 >> 2, (k8 >> 1) & 1, k8 & 1
                        rhs = imgz[:, cc, kh::2, kw::2]
                        nc.tensor.matmul(zps, wsr_t[:, k8, mt * 128:(mt + 1) * 128],
                                         rhs, start=(k8 == 0), stop=(k8 == 7))
                    nc.scalar.activation(y16[:, mt, 0:LZ], zps, AF.Identity,
                                         bias=lnp_t[:, mt, 0:1])
                else:
                    xt = part
                    xps = pre_tile()
                    for k8 in range(8):
                        kh, kw, cc = k8 >> 2, (k8 >> 1) & 1, k8 & 1
                        rhs = imgx[:, cc, 32 * xt + kh: 32 * xt + kh + 31: 2, kw::2]
                        nc.tensor.matmul(xps, wsr_t[:, k8, mt * 128:(mt + 1) * 128],
                                         rhs, start=(k8 == 0), stop=(k8 == 7))
                    nc.scalar.activation(y16[:, mt, LZ + QTILE * xt: LZ + QTILE * (xt + 1)],
                                         xps, AF.Identity, bias=lnp_t[:, mt, 0:1])

        # ---- layernorm over channels (stats via fp16 ones-matmul) ----
        ysq16 = pre_sb.tile([128, 2, L], F16, tag="ysq")
        for cc in range(2):
            nc.vector.tensor_mul(ysq16[:, cc, :], y16[:, cc, :], y16[:, cc, :])
        mean_b = pre_sb.tile([128, L], F32, tag="mean")
        var_b = pre_sb.tile([128, L], F32, tag="var")
        for off, sz in ((0, 512), (512, 512), (1024, 256)):
            s1_ps = pre_tile()
            s2_ps = pre_tile()
            for cc in range(2):
                nc.tensor.matmul(s1_ps[:, :sz], onesC,
                                 y16[:, cc, off:off + sz],
                                 start=(cc == 0), stop=(cc == 1))
                nc.tensor.matmul(s2_ps[:, :sz], onesC,
                                 ysq16[:, cc, off:off + sz],
                                 start=(cc == 0), stop=(cc == 1))
            nc.scalar.copy(mean_b[:, off:off + sz], s1_ps[:, :sz])
            nc.scalar.copy(var_b[:, off:off + sz], s2_ps[:, :sz])
        msq_b = pre_sb.tile([128, L], F32, tag="msq")
        nc.vector.tensor_mul(msq_b, mean_b, mean_b)
        std_b = pre_sb.tile([128, L], F32, tag="std")
        rstd_b = pre_sb.tile([128, L], F32, tag="rstd")
        rscr_b = pre_sb.tile([128, L], F32, tag="rscr")
        nc.vector.tensor_tensor(var_b, var_b, msq_b, OP.subtract)
        nc.scalar.activation(std_b, var_b, AF.Sqrt, bias=eps_t[:, 0:1])
        nc.vector.reciprocal_approx_accurate(rstd_b, std_b, rscr_b)
        for cc in range(2):
            t32 = pre_sb.tile([128, L], F32, tag="t32")
            nc.vector.tensor_tensor(t32, y16[:, cc, :].bitcast(F16), mean_b, OP.subtract)
            nc.vector.tensor_tensor(t32, t32, rstd_b, OP.mult)
            nc.scalar.activation(catn16[:, cc, :], t32, AF.Identity,
                                 bias=lnp_t[:, cc, 2:3],
                                 scale=lnp_t[:, cc, 1:2])

        # ---- K^T and V (fp16) ----
        for off, sz in ((0, 512), (512, 512), (1024, 256)):
            kps = pre_tile()
            for cc in range(2):
                nc.tensor.matmul(kps[:, :sz], wk_t[:, cc, :],
                                 catn16[:, cc, off:off + sz],
                                 start=(cc == 0), stop=(cc == 1))
            if PRE_SPLIT and off == 512:
                nc.vector.tensor_copy(kt16[:, off:off + sz], kps[:, :sz])
            else:
                nc.scalar.copy(kt16[:, off:off + sz], kps[:, :sz])
        for jt0 in range(0, NJT, 4):
            nv = min(4, NJT - jt0)
            vps = ps.tile([128, 2, QTILE], F32, tag="s", name="vps")
            vflat = vps.rearrange("p c q -> p (c q)")
            for j4 in range(nv):
                jt = jt0 + j4
                for cc in range(2):
                    nc.tensor.matmul(vflat[:, j4 * 128:(j4 + 1) * 128],
                                     catn16[:, cc, jt * 128:(jt + 1) * 128],
                                     wv_t[:, cc, :],
                                     start=(cc == 0), stop=(cc == 1))
            if PRE_SPLIT and jt0 == 4:
                nc.vector.tensor_copy(
                    v16[:, jt0:jt0 + nv, :].rearrange("p j m -> p (j m)"),
                    vflat[:, :nv * 128])
            else:
                nc.scalar.copy(
                    v16[:, jt0:jt0 + nv, :].rearrange("p j m -> p (j m)"),
                    vflat[:, :nv * 128])

    # ---- attention + projection, one 512-query tile at a time ----
    if do_attn:
        p16c = None
        if EXPF == "mini":
            p16c = pre_sb.tile([128, 2, QTILE], F16, tag="p16c")
            nc.gpsimd.memset(p16c, 0.00078125)

        def emit_tail(t):
            o_t, den_t, qt_t = t
            recip = w_pool.tile([128, QTILE], F32, tag="recip")
            nc.vector.reciprocal_approx_fast(recip, den_t)
            onorm = w_pool.tile([128, QTILE], F32, tag="onorm")
            nc.vector.tensor_mul(onorm, o_t, recip)
            for mt in range(2):
                if DEFER_TAIL:
                    pps = ps.tile([128, 2, QTILE], F32, tag="s", name="pps")[:, 0, :]
                else:
                    pps = acc_ps_pool.tile([128, QTILE], F32, tag="acc", name="pps")
                nc.tensor.matmul(pps, wp_t[:, mt * 128:(mt + 1) * 128], onorm,
                                 start=True, stop=True)
                ysb = w_pool.tile([128, QTILE], F32, tag="ysb")
                if PSPLIT and mt == 1:
                    nc.vector.tensor_copy(ysb, pps)
                else:
                    nc.scalar.copy(ysb, pps)
                nc.sync.dma_start(
                    out=yT_d[mt * 128:(mt + 1) * 128,
                             qt_t * QTILE:(qt_t + 1) * QTILE],
                    in_=ysb)
            if DEFER_TAIL and TAIL_DUMMY:
                ps.tile([128, 2, QTILE], F32, tag="s", name="sdummy")

        tail = None
        for qt in range(NQT):
            jts = list(range(ZJT)) if qt < ZQT else list(range(NJT))
            o_ps = acc_ps_pool.tile([128, QTILE], F32, tag="acc")
            den_ps = acc_ps_pool.tile([128, QTILE], F32, tag="acc")

            def emit_zeros():
                if MS_ZERO == "dve":
                    nc.vector.memset(o_ps, 0.0)
                    nc.vector.memset(den_ps, 0.0)
                elif MS_ZERO == "act":
                    nc.scalar.copy(o_ps, zero32)
                    nc.scalar.copy(den_ps, zero32)
                else:
                    nc.tensor.matmul(o_ps, ones16, zero16, start=True,
                                     stop=True, skip_group_check=True)
                    nc.tensor.matmul(den_ps, ones16, zero16, start=True,
                                     stop=True, skip_group_check=True)

            if not DEFER_TAIL:
                emit_zeros()
            qsl = slice(qt * QTILE, (qt + 1) * QTILE)
            prevs_a, prevs_b = [], []

            def emit_pair(p2, jt2, h0):
                for hh in range(2):
                    h = h0 + hh
                    src = p16c[:, hh, :] if EXPF == "mini" else p2[:, hh, :]
                    nc.tensor.matmul(o_ps[32 * h:32 * h + 32, :],
                                     v16[:, jt2, 32 * h:32 * h + 32],
                                     src,
                                     start=False, stop=False,
                                     tile_position=(0, 32 * h),
                                     skip_group_check=True)
                if NODEN:
                    return
                for hh in range(2):
                    h = h0 + hh
                    src = p16c[:, hh, :] if EXPF == "mini" else p2[:, hh, :]
                    nc.tensor.matmul(den_ps[32 * h:32 * h + 32, :],
                                     ones16[:, 0:32], src,
                                     start=False, stop=False,
                                     tile_position=(0, 32 * h),
                                     skip_group_check=True)

            def emit_odens(pa, pb):
                for h in range(4):
                    pp = pa if h < 2 else pb
                    src = (p16c[:, h % 2, :] if EXPF == "mini"
                           else pp[0][:, h % 2, :])
                    nc.tensor.matmul(o_ps[32 * h:32 * h + 32, :],
                                     v16[:, pp[1], 32 * h:32 * h + 32],
                                     src, start=False, stop=False,
                                     tile_position=(0, 32 * h),
                                     skip_group_check=True)
                if NODEN:
                    return
                for h in range(4):
                    pp = pa if h < 2 else pb
                    src = (p16c[:, h % 2, :] if EXPF == "mini"
                           else pp[0][:, h % 2, :])
                    nc.tensor.matmul(den_ps[32 * h:32 * h + 32, :],
                                     ones16[:, 0:32], src,
                                     start=False, stop=False,
                                     tile_position=(0, 32 * h),
                                     skip_group_check=True)

            for jt in jts:
                ksl = slice(jt * 128, (jt + 1) * 128)
                sa = ps.tile([128, 2, QTILE], F32, tag="s")
                for h in range(2):
                    nc.tensor.matmul(sa[:, h, :], kt16[32 * h:32 * h + 32, ksl],
                                     qt16[32 * h:32 * h + 32, qsl],
                                     start=True, stop=True,
                                     tile_position=(32 * h, 0))
                pa = p_pool.tile([128, 2, QTILE], F16, tag="pa")
                if EXPF == "mini":
                    nc.scalar.activation(pa[:, :, 0:8], sa[:, :, 0:8],
                                         AF.Exp, scale=SCALE)
                else:
                    nc.scalar.activation(pa, sa, AF.Exp, scale=SCALE)
                if EMIT_MID and len(prevs_a) >= ODEN_LAG:
                    pp = prevs_a.pop(0)
                    emit_pair(pp[0], pp[1], 0)
                    if prevs_b:
                        pp = prevs_b.pop(0)
                        emit_pair(pp[0], pp[1], 2)
                sb = ps.tile([128, 2, QTILE], F32, tag="s")
                for h in range(2, 4):
                    nc.tensor.matmul(sb[:, h - 2, :], kt16[32 * h:32 * h + 32, ksl],
                                     qt16[32 * h:32 * h + 32, qsl],
                                     start=True, stop=True,
                                     tile_position=(32 * h, 0))
                pb = p_pool.tile([128, 2, QTILE], F16, tag="pb", bufs=LAG_B + 2)
                if EXPF == "mini":
                    nc.vector._custom_dve(EXP_OP, out=pb[:, :, 0:8], in0=sb[:, :, 0:8],
                                          in1=ec4_t, s0=EXP_C1, s1=EXP_C2,
                                          imm2=EXP_C3)
                else:
                    nc.vector._custom_dve(EXP_OP, out=pb, in0=sb,
                                          in1=ec4_t, s0=EXP_C1, s1=EXP_C2,
                                          imm2=EXP_C3)
                prevs_a.append((pa, jt))
                prevs_b.append((pb, jt))
                if DEFER_TAIL and jt == jts[1]:
                    if tail is not None:
                        emit_tail(tail)
                    emit_zeros()
                if not EMIT_MID:
                    if LAG_B == ODEN_LAG:
                        if len(prevs_a) > ODEN_LAG:
                            pa2 = prevs_a.pop(0)
                            pb2 = prevs_b.pop(0)
                            emit_odens(pa2, pb2)
                    else:
                        if len(prevs_a) > ODEN_LAG:
                            pp = prevs_a.pop(0)
                            emit_pair(pp[0], pp[1], 0)
                        if len(prevs_b) > LAG_B:
                            pp = prevs_b.pop(0)
                            emit_pair(pp[0], pp[1], 2)
            if LAG_B == ODEN_LAG:
                for pa2, pb2 in zip(prevs_a, prevs_b):
                    emit_odens(pa2, pb2)
            else:
                for pp in prevs_a:
                    emit_pair(pp[0], pp[1], 0)
                for pp in prevs_b:
                    emit_pair(pp[0], pp[1], 2)

            if DEFER_TAIL:
                tail = (o_ps, den_ps, qt)
            else:
                emit_tail((o_ps, den_ps, qt))
        if DEFER_TAIL and tail is not None:
            emit_tail(tail)


def _get_nc():
    if "nc" not in _CACHED:
        _CACHED["nc"] = _build_nc()
    return _CACHED["nc"]


def _prep_inputs(x, Wq, Wkv, Wsr, bsr, gamma, beta, Wproj, bproj):
    """Build the 8 per-core input dicts (host-side shard + transpose)."""
    x = np.asarray(x, np.float32)
    Wq = np.asarray(Wq, np.float32)
    Wkv = np.asarray(Wkv, np.float32)
    Wsr = np.asarray(Wsr, np.float32)
    Wproj = np.asarray(Wproj, np.float32)
    lnp = np.ascontiguousarray(
        np.stack([np.asarray(bsr, np.float32), np.asarray(gamma, np.float32),
                  np.asarray(beta, np.float32)], axis=1))
    wsr8 = np.ascontiguousarray(
        Wsr.transpose(2, 3, 1, 0).reshape(8, 128, C).astype(np.float32))
    in_maps = []
    for core in range(NCORES):
        b, hg = core // 2, core % 2
        sl = slice(hg * 128, (hg + 1) * 128)
        in_maps.append({
            "xT": np.ascontiguousarray(x[b].T.astype(np.float16)),
            "wq": np.ascontiguousarray(Wq[sl, :].T.astype(np.float16)),
            "wsr": wsr8.astype(np.float16),
            "wk": np.ascontiguousarray(Wkv[:C][sl, :].T.astype(np.float16)),
            "wv": np.ascontiguousarray(Wkv[C:][sl, :].T.astype(np.float16)),
            "wp": np.ascontiguousarray(Wproj[:, sl].T),
            "lnp": lnp,
        })
    return in_maps


def kernel(x, Wq, Wkv, Wsr, bsr, gamma, beta, Wproj, bproj,
           H_x=64, W_x=64, H_z=32, W_z=32, _trace=False, _trace_kwargs=None):
    assert int(H_x) == HX and int(W_x) == WX and int(H_z) == HZ and int(W_z) == WZ
    nc = _get_nc()
    in_maps = _prep_inputs(x, Wq, Wkv, Wsr, bsr, gamma, beta, Wproj, bproj)
    kw = dict(_trace_kwargs or {})
    res = run_bass_kernel_spmd(nc, in_maps, core_ids=list(range(NCORES)),
                               trace=_trace, **kw)
    _CACHED["last_result"] = res
    bproj = np.asarray(bproj, np.float32)
    out = np.empty((B, N, C), np.float32)
    for b in range(B):
        yT = res.results[2 * b]["yT"] + res.results[2 * b + 1]["yT"]
        out[b] = yT.T + bproj
    return out
